# revision 50
# baseline (speedup 1.0000x reference)
"""Bass/Trainium2 kernel for nn_Attention_10299331576042.

Math: reference computes
    energies = enc @ W.T + b          # [S, H]
    scores   = energies @ hidden      # [S]
    attn     = softmax(scores)        # [1, 1, S]

Algebra: scores = enc @ (hidden @ W) + (b . hidden).  The (b . hidden) term is
a constant shift across the sequence axis, and softmax is shift-invariant, so
it drops out exactly.  The problem reduces to a memory-bound matvec
    v = hidden @ W                    # [H]      (tiny)
    scores = enc @ v                  # [S]      (reads all 128 MiB of enc)
followed by a softmax over S = 32768 scores.

Sharding: enc is split along seq_len across the 8 NeuronCores (16 MiB each);
hidden and W are replicated.  Launch 1 (8 cores): W streams in 8 chunks while
PE "filler" matmuls hold the clock at full p-state, so v is accumulated the
moment the last chunk lands; the enc shard then streams through one fused
DVE op per row (scalar_tensor_tensor: in-place multiply by v with the row
sum accumulated straight into the scores tile) at the DMA cadence.  Launch 2
(8 cores): every core receives the full score vector (rotated so its own
shard sits at the front) plus a host identity matrix, computes the global
max via a PE transpose, the row-stable exp, Z via one PE dot product
(z . exp(m - M)), and writes only its 4096-element shard of attn.

The walrus build in this container supports only ONE sync wait per
instruction and cannot codegen InstISA ops.  Consequences baked in here:
  - only classic BIR instructions; scalar_tensor_tensor (InstTensorScalarPtr)
    is the one fused op available, and only on DVE (the Pool engine variant
    is rejected by walrus -- Pool handles two rows as plain multiplies with
    ACT activation+accum reduces instead),
  - enc supertiles and W chunks never reuse SBUF slots (no WAW/WAR waits on
    DMAs); all loads share one HWDGE ring, the scores store uses the idle
    SWDGE ring,
  - tiny "absorber" copies let an engine observe a producer once so later
    dependencies merge onto a single semaphore; readers of one tile on
    DIFFERENT engines get serialized pairwise by the framework, so Pool
    works from its own SBUF copy of v,
  - partition reductions/broadcasts use PE matmuls (rank-1 tricks and
    is_transpose with an identity fed from the host).
"""

from contextlib import ExitStack

import numpy as np

import concourse.bass as bass
import concourse.tile as tile
from concourse import mybir
from concourse.bass_utils import run_bass_kernel_spmd
from concourse.vector_clock import ScopedClock


class _SplitDrainTileContext(tile.TileContext):
    """TileContext whose kernel-tail drain is split into single-wait drains.

    The walrus build in this container rejects any instruction carrying more
    than one sync wait; the stock tail drain waits on every semaphore at once.
    A chain of drains, each waiting on one semaphore, is semantically
    identical (all waits complete before the end-of-kernel barrier).
    """

    def _drain_and_barrier(self, tick_clock, wait_clock):
        drain_inst = self.nc.sync.drain()
        wait_clock.add_sem_waits(
            drain_inst.ins, ScopedClock({None: tick_clock.global_clock})
        )
        si = drain_inst.ins.sync_info
        waits = list(si.on_wait) if si is not None and si.on_wait else []
        if len(waits) > 1:
            drain_inst.ins.sync_info = mybir.SyncInfo(
                on_wait=[waits[0]],
                on_update=list(si.on_update) if si.on_update else [],
            )
            for w in waits[1:]:
                extra = self.nc.sync.drain().ins
                extra.sync_info = mybir.SyncInfo(on_wait=[w], on_update=[])

        self.nc.all_engine_barrier()
        assert self.sems is not None
        popped = self.nc._tile_sem_poison_stack.pop()
        assert popped is self._sem_poison
        self.nc.clear_and_free_semaphores(list(self.sems.allocated().values()))
        self.nc.all_engine_barrier()

N_CORES = 8
S = 32768
H = 1024
SS = S // N_CORES          # 4096 rows per core
P = 128                    # partitions
RPP = SS // P              # 32 rows per partition
# Supertile row counts: 2-row tiles while DMA streams, then 1-row tiles so
# the tail work (mul + reduce per row) pipelines across DVE/ACT/Pool at the
# 1456ns DMA cadence and the post-DMA tail is one small mul + reduce.
TILES = [2] * 13 + [1] * 6
F32 = mybir.dt.float32

TRACE = False
LAST_PERF = {}

_NC_CACHE = {}


def _reduce_pending(nc, pending, scores_sb):
    """ACT-side reduce of a DVE/Pool-produced product row.

    The activation reduces the row in place with its sum accumulated into
    scores_sb[:, col]; its single sync wait is the producing engine's
    semaphore.
    """
    prod_row, col = pending
    nc.scalar.activation(
        out=prod_row,
        in_=prod_row,
        func=mybir.ActivationFunctionType.Copy,
        accum_out=scores_sb[:, col:col + 1],
    )


def _build_scores_nc():
    """Per-core kernel: scores_shard[4096] = enc_shard @ (hidden @ W)."""
    nc = bass.Bass("TRN2", target_bir_lowering=False, debug=False)
    enc = nc.dram_tensor("enc", [SS, H], F32, kind="ExternalInput").ap()
    hid = nc.dram_tensor("hidden", [H], F32, kind="ExternalInput").ap()
    w = nc.dram_tensor("w", [H, H], F32, kind="ExternalInput").ap()
    scores = nc.dram_tensor("scores", [SS], F32, kind="ExternalOutput").ap()

    enc3 = enc.rearrange("(p i) h -> p i h", p=P)  # [128, 32, 1024]
    nd = H // P  # 8 W chunks

    with _SplitDrainTileContext(nc) as tc, ExitStack() as ctx:
        singles = ctx.enter_context(tc.tile_pool(name="singles", bufs=1))
        n_big = sum(1 for r in TILES if r == 2)
        n_small = len(TILES) - n_big
        stpool = ctx.enter_context(tc.tile_pool(name="stpool", bufs=n_big))
        stpool2 = ctx.enter_context(tc.tile_pool(name="stpool2", bufs=n_small + 1))
        wpool = ctx.enter_context(tc.tile_pool(name="wpool", bufs=nd))
        dpool = ctx.enter_context(tc.tile_pool(name="dpool", bufs=len(TILES)))
        psum = ctx.enter_context(tc.tile_pool(name="psum", bufs=1, space="PSUM"))

        # ---- v_rep = (hidden @ W) replicated on all partitions, in PSUM ----
        # hid_sb[p, c] = hidden[c*128 + p]
        hid_sb = singles.tile([P, nd], F32)
        nc.sync.dma_start(out=hid_sb, in_=hid.rearrange("(c p) -> p c", p=P))
        # DVE absorber for the hid DMA, then broadcast hidden along the free
        # dim: hid_rep3[p, c, m] = hidden[c*128 + p] for all m.
        junk0 = singles.tile([P, 2], F32)
        nc.vector.tensor_copy(out=junk0, in_=hid_sb[:, 0:2])
        hid_rep3 = singles.tile([P, nd, P], F32)
        nc.vector.memset(hid_rep3, 0.0)
        for c in range(nd):
            nc.vector.tensor_scalar_add(
                out=hid_rep3[:, c, :],
                in0=hid_rep3[:, c, :],
                scalar1=hid_sb[:, c:c + 1],
            )
        # PE absorber: take the DVE (hid_sb) wait so the matmuls below only
        # wait on their W chunk's DMA lane.  The filler matmuls that follow
        # keep the PE continuously busy through the W stream: the cost model
        # ramps the PE clock up only after ~3us without an idle gap, and a
        # PE stalled waiting on each W chunk never leaves the mid p-state.
        ptiny = psum.tile([1, 8], F32, tag="tiny")
        nc.tensor.matmul(
            ptiny[:, 0:1],
            lhsT=hid_sb[:, 0:1],
            rhs=hid_sb[:, 0:1],
            start=True,
            stop=True,
        )
        for _ in range(64):
            nc.tensor.matmul(
                ptiny,
                lhsT=hid_sb[:, 0:1],
                rhs=hid_sb,
                start=True,
                stop=True,
            )
        # absorb the DVE (hid_rep3) tick so the W matmuls only wait on DMA
        nc.tensor.matmul(
            ptiny[:, 0:1],
            lhsT=hid_rep3[:, nd - 1, 0:1],
            rhs=hid_rep3[:, nd - 1, 0:1],
            start=True,
            stop=True,
        )
        # W streamed in 8 chunks (separate slots); both 512-wide halves of v
        # accumulate right after each chunk lands (chunk-outer order) so v is
        # ready as soon as the last chunk arrives -- the PE ramps to full
        # clock during the W stream instead of after it.
        psum_vrep = psum.tile([P, H], F32, tag="vrep")
        w_sbs = []
        for c in range(nd):
            w_sb = wpool.tile([P, H], F32, tag="w")
            nc.sync.dma_start(out=w_sb, in_=w[c * P:(c + 1) * P, :])
            w_sbs.append(w_sb)
            for half in range(2):
                nc.tensor.matmul(
                    psum_vrep[:, half * 512:(half + 1) * 512],
                    lhsT=hid_rep3[:, c, :],
                    rhs=w_sb[:, half * 512:(half + 1) * 512],
                    start=(c == 0),
                    stop=(c == nd - 1),
                )
        # ---- enc supertile loads: zero-wait DMAs behind the W stream ----
        sts = []
        row = 0
        for t, rpt in enumerate(TILES):
            pool_t = stpool if rpt == 2 else stpool2
            st = pool_t.tile([P, rpt, H], F32, tag="st", name=f"st{t}")
            nc.sync.dma_start(out=st, in_=enc3[:, row:row + rpt, :])
            sts.append((st, row, rpt))
            row += rpt

        # ---- scores = enc_shard @ v ----
        # Row layout: local row s = p*32 + i  ->  scores_sb[p, i]
        # One fused DVE op per row (scalar_tensor_tensor, classic
        # InstTensorScalarPtr): out = (st * 1.0) * v in place, with the row
        # sum accumulated straight into scores_sb[:, i].  No second engine
        # touches the stream, so the only cross-engine sems are the DMA
        # absorbers and the PE (v) absorber.
        scores_sb = singles.tile([P, RPP], F32)
        # DVE absorber for the PE (v) semaphore so mul0 only waits on DMA.
        junk_v = singles.tile([P, 2], F32)
        nc.vector.tensor_copy(out=junk_v, in_=psum_vrep[:, 0:2])
        v_sb2 = singles.tile([P, H], F32)
        pool_rows = {13, 14}   # first two 1-row tiles run on the idle Pool
        for t, (st, row, rpt) in enumerate(sts):
            if t == 2:
                # DVE's early slack: make Pool's SBUF copy of v now (Pool
                # must not share the PSUM v with the DVE stream -- readers
                # on different engines get serialized pairwise).
                nc.vector.tensor_copy(out=v_sb2, in_=psum_vrep)
                junk_pv = singles.tile([P, 2], F32)
                nc.gpsimd.tensor_copy(out=junk_pv, in_=v_sb2[:, 0:2])
            if t in pool_rows:
                # walrus rejects scalar_tensor_tensor on Pool; plain in-place
                # multiply there, with the row reduces on the otherwise-idle
                # ACT (activation Copy + accum_out, waits Pool's semaphore).
                vb2 = bass.AP(
                    tensor=v_sb2.tensor,
                    offset=v_sb2.offset,
                    ap=[list(v_sb2.ap[0]), [0, rpt], list(v_sb2.ap[1])],
                )
                nc.gpsimd.tensor_mul(st, st, vb2)
                for j in range(rpt):
                    _reduce_pending(nc, (st[:, j, :], row + j), scores_sb)
                continue
            # DVE absorber for this supertile's DMA lane
            junk = dpool.tile([P, 2], F32, tag="junk")
            nc.vector.tensor_copy(out=junk, in_=st[:, 0, 0:2])
            for j in range(rpt):
                nc.vector.scalar_tensor_tensor(
                    out=st[:, j, :],
                    in0=st[:, j, :],
                    scalar=1.0,
                    in1=psum_vrep,
                    op0=mybir.AluOpType.mult,
                    op1=mybir.AluOpType.mult,
                    accum_out=scores_sb[:, row + j:row + j + 1],
                )
        # Pool already observed ACT's reduces of its own rows (junk_pa, issued
        # mid-stream while Pool idles), so the SWDGE store's single wait is
        # the DVE semaphore for the final fused row.
        junk_pa = singles.tile([P, 2], F32)
        nc.gpsimd.tensor_copy(out=junk_pa, in_=scores_sb[:, 26:28])
        nc.gpsimd.dma_start(out=scores.rearrange("(p i) -> p i", p=P), in_=scores_sb)
    return nc


def _build_softmax_nc():
    """8-core SPMD softmax: every core gets the full scores vector rotated so
    its own 4096-row shard sits at positions [0, 4096); it computes the
    global max / sum and writes only its shard of attn.

    Cross-partition steps use PE matmuls: an is_transpose matmul against a
    host-fed identity turns per-partition scalars into a row on partition 0
    (and back).
    """
    nc = bass.Bass("TRN2", target_bir_lowering=False, debug=False)
    scores = nc.dram_tensor("scores", [S], F32, kind="ExternalInput").ap()
    iden = nc.dram_tensor("iden", [P, P], F32, kind="ExternalInput").ap()
    attn = nc.dram_tensor("attn", [SS], F32, kind="ExternalOutput").ap()
    FD = S // P   # 256
    SHP = SS // FD  # 16 partitions hold this core's shard

    with _SplitDrainTileContext(nc) as tc, ExitStack() as ctx:
        pool = ctx.enter_context(tc.tile_pool(name="p", bufs=1))
        psum = ctx.enter_context(tc.tile_pool(name="ps", bufs=1, space="PSUM"))
        sc = pool.tile([P, FD], F32)
        nc.sync.dma_start(out=sc, in_=scores.rearrange("(p j) -> p j", p=P))
        idsb = pool.tile([P, P], F32)
        nc.sync.dma_start(out=idsb, in_=iden)
        ones_r = pool.tile([1, P], F32)
        nc.vector.memset(ones_r, 1.0)

        # per-partition max and its negation (exp bias)
        m1 = pool.tile([P, 1], F32)
        nc.vector.reduce_max(m1, sc, axis=mybir.AxisListType.X)
        nm1 = pool.tile([P, 1], F32)
        nc.vector.tensor_scalar_mul(out=nm1, in0=m1, scalar1=-1.0)

        # ACT absorber for the scores DMA, then the row-stable exp:
        # e[p, j] = exp(sc[p, j] - m_p), z[p] = sum_j e[p, j]
        junk_a = pool.tile([P, 2], F32)
        nc.scalar.copy(out=junk_a, in_=sc[:, 0:2])
        e = pool.tile([P, FD], F32)
        z = pool.tile([P, 1], F32)
        nc.scalar.activation(
            out=e,
            in_=sc,
            func=mybir.ActivationFunctionType.Exp,
            bias=nm1,
            scale=1.0,
            accum_out=z,
        )

        # PE absorber for the identity DMA, then transpose the per-partition
        # maxima into a row: mt_ps[0, p] = m_p.
        ptiny = psum.tile([1, 2], F32, tag="tiny")
        nc.tensor.matmul(
            ptiny[:, 0:1], lhsT=idsb[:, 0:1], rhs=idsb[:, 0:1], start=True, stop=True
        )
        mt_ps = psum.tile([1, P], F32, tag="mt")
        nc.tensor.transpose(mt_ps, m1, idsb)

        # -M on partition 0, broadcast back to a column, then the
        # per-partition corrections t_p = exp(m_p - M) on ACT.
        negM = pool.tile([1, 1], F32)
        nc.vector.reduce_max(negM, mt_ps, axis=mybir.AxisListType.X, negate=True)
        negm_ps = psum.tile([P, 1], F32, tag="negm")
        nc.tensor.matmul(negm_ps, lhsT=ones_r, rhs=negM, start=True, stop=True)
        nmc = pool.tile([P, 1], F32)
        nc.scalar.copy(out=nmc, in_=negm_ps)
        t_col = pool.tile([P, 1], F32)
        nc.scalar.activation(
            out=t_col,
            in_=m1,
            func=mybir.ActivationFunctionType.Exp,
            bias=nmc,
            scale=1.0,
        )
        # shard numerator (independent of Z): a1 = e * t on the shard rows
        a1 = pool.tile([SHP, FD], F32)
        nc.scalar.activation(
            out=a1,
            in_=e[0:SHP, :],
            func=mybir.ActivationFunctionType.Copy,
            scale=t_col[0:SHP],
        )

        # Z = sum_p z_p t_p via one PE dot product, 1/Z back to a column
        z_ps = psum.tile([1, 1], F32, tag="z")
        nc.tensor.matmul(z_ps, lhsT=z, rhs=t_col, start=True, stop=True)
        rz = pool.tile([1, 1], F32)
        nc.vector.reciprocal(rz, z_ps)
        rzb_ps = psum.tile([P, 1], F32, tag="rzb")
        nc.tensor.matmul(rzb_ps, lhsT=ones_r, rhs=rz, start=True, stop=True)
        rzc = pool.tile([P, 1], F32)
        nc.scalar.copy(out=rzc, in_=rzb_ps)
        a16 = pool.tile([SHP, FD], F32)
        nc.scalar.activation(
            out=a16,
            in_=a1,
            func=mybir.ActivationFunctionType.Copy,
            scale=rzc[0:SHP],
        )
        nc.sync.dma_start(out=attn.rearrange("(p j) -> p j", p=SHP), in_=a16)
    return nc


def _get_nc(name, builder):
    if name not in _NC_CACHE:
        _NC_CACHE[name] = builder()
    return _NC_CACHE[name]


_IDEN = np.eye(P, dtype=np.float32)


def kernel(hidden, encoder_outputs, W, b):
    hidden = np.ascontiguousarray(np.asarray(hidden, dtype=np.float32))
    enc = np.ascontiguousarray(np.asarray(encoder_outputs, dtype=np.float32))
    W = np.ascontiguousarray(np.asarray(W, dtype=np.float32))
    # b drops out of softmax (constant shift across seq_len)

    nc_scores = _get_nc("scores", _build_scores_nc)
    in_maps = [
        {
            "enc": np.ascontiguousarray(enc[k * SS:(k + 1) * SS]),
            "hidden": hidden,
            "w": W,
        }
        for k in range(N_CORES)
    ]
    res = run_bass_kernel_spmd(
        nc_scores, in_maps, core_ids=list(range(N_CORES)), trace=TRACE
    )
    LAST_PERF["scores"] = res
    scores = np.concatenate([res.results[k]["scores"] for k in range(N_CORES)])

    nc_soft = _get_nc("softmax", _build_softmax_nc)
    in_maps2 = [
        {"scores": np.ascontiguousarray(np.roll(scores, -k * SS)), "iden": _IDEN}
        for k in range(N_CORES)
    ]
    res2 = run_bass_kernel_spmd(
        nc_soft, in_maps2, core_ids=list(range(N_CORES)), trace=TRACE
    )
    LAST_PERF["softmax"] = res2
    attn = np.concatenate([res2.results[k]["attn"] for k in range(N_CORES)])

    return np.asarray(attn, dtype=np.float32).reshape(1, 1, S)
